# revision 10
# baseline (speedup 1.0000x reference)
"""AGNNConv distributed Bass kernel for 8 TRN2 NeuronCores (v8).

out = (1+eps)*feat + h,  h[d] = sum_{e: dst_e=d} p_e * norm_feat[src_e]
with p_e = edge-softmax grouped by src.

Algebra:
    w_e = exp(beta*ew_e)
    z_n = sum_{e: src_e=n} w_e            # per NODE
    g_n = feat_n / (||feat_n|| * z_n)     # per NODE
    h_d = sum_{e: dst_e=d} w_e * g[src_e]
    out = (1+eps)*feat + h

v8 (v7 lost ~40us to group-boundary bubbles: ACT exp waited on the
strided straw DMA, PE waited on ACT, out-DMAs were 32KB):
  Phase 1 (node-sharded): g64 = 64*g (bf16 -> host casts fp8) and
    o0 = (1+eps)*feat per node. o0 on ScalarE; beta/eps replicated
    to [128,1] on host so no GpSimd broadcasts.
  Host: gathers g64[src_e] per edge; scatters RAW ew_e values (pure
    relayout) into one-hot slots of fp8 straw with -80 fill. Both
    per-edge streams stored GROUP-CONTIGUOUS in DRAM (4 dst tiles
    padded to GMAX edge tiles) so each group is one linear read.
  Phase 2: per group: straw DMA first, then ge; one batched ScalarE
    exp builds stw[e,d] = exp(beta*straw - ln64) = w_e/64 one-hot
    placed; h-tile = stw^T @ ge64 as PSUM-accumulating matmuls;
    ot = o0 + hp per tile into a group buffer, one out DMA per group.
"""

import sys

sys.path.insert(0, "/opt/trn_rl_repo")

import numpy as np

N, E, D = 50000, 640000, 128
NCORES = 8
SH = N // NCORES            # 6250 dst nodes per core
HTILES = (SH + 127) // 128  # 49 dst tiles per core
SHP = HTILES * 128          # 6272 padded nodes per core

PAD_EW = -80.0              # exp(beta*PAD_EW) == 0 (inside ACT LUT range)
LN64 = 4.1588830833596715
GRP = 4                     # dst tiles per DMA/ACT batch
NG = (HTILES + GRP - 1) // GRP  # 13 groups
# groups whose w-placement runs on DVE (onehot/64 fp8 + msgq=ge*wv)
# instead of ScalarE exp-in-place; balances the two engines (~48us each)
DVE_GROUPS = frozenset({1, 4, 7, 9, 11})


def _host_prep(src, dst, edge_weight):
    """Index/layout prep only (no float math on tensor values)."""
    import ml_dtypes

    src = np.asarray(src).astype(np.int64)
    dst = np.asarray(dst).astype(np.int64)
    ew = np.asarray(edge_weight).astype(np.float32)

    # ---- per-node src-grouped edge-weight rows (for z), fixed K ----
    deg = np.bincount(src, minlength=N)
    K = int(deg.max())
    order = np.argsort(src, kind="stable")
    starts = np.zeros(N + 1, np.int64)
    np.cumsum(deg, out=starts[1:])
    slot = np.arange(E, dtype=np.int64) - starts[src[order]]
    zpad = np.full((N, K), PAD_EW, np.float32)
    zpad[src[order], slot] = ew[order]

    # per-core phase-1 z rows: [128, HTILES*K] bf16
    zrows = []
    for c in range(NCORES):
        zm = np.full((SHP, K), PAD_EW, np.float32)
        zm[:SH] = zpad[c * SH : (c + 1) * SH]
        zrows.append(
            np.ascontiguousarray(
                zm.reshape(HTILES, 128, K).transpose(1, 0, 2)
            ).reshape(128, HTILES * K).astype(ml_dtypes.bfloat16)
        )

    # ---- per-core edge grouping by dst tile ----
    owner = dst // SH
    dstl = dst - owner * SH
    dtile = dstl // 128
    dbit = dstl % 128

    counts = np.zeros((NCORES, HTILES), np.int64)
    np.add.at(counts, (owner, dtile), 1)
    net = (counts.max(axis=0) + 127) // 128  # [HTILES] edge tiles per dst tile
    net = np.maximum(net, 1)

    # group-local layout: tiles of group g at local cumsum offsets,
    # each group padded to GMAX edge tiles
    loff = np.zeros(HTILES, np.int64)   # edge-tile offset of dst tile
    gsum = np.zeros(NG, np.int64)       # edge tiles per group
    for g in range(NG):
        o = 0
        for i in range(g * GRP, min((g + 1) * GRP, HTILES)):
            loff[i] = o
            o += int(net[i])
        gsum[g] = o
    GMAX = int(gsum.max())
    EPAD = NG * GMAX * 128

    core_idx = []
    for c in range(NCORES):
        m = np.nonzero(owner == c)[0]
        key = dtile[m]
        korder = np.argsort(key, kind="stable")
        me = m[korder]
        keys = key[korder]
        kb = np.r_[0, np.nonzero(np.diff(keys))[0] + 1]
        sf = np.zeros(len(keys), np.int64)
        sf[kb] = kb
        np.maximum.accumulate(sf, out=sf)
        within = np.arange(len(keys)) - sf
        grp = keys // GRP
        pos = (grp * GMAX + loff[keys]) * 128 + within

        src_pad = np.full(EPAD, N, np.int64)      # pad edges read zero row N

        # scatter matrix, layout [NG*128, GMAX*128], group-contiguous.
        # ACT groups: RAW ew_e at [e, dbit_e], -80 fill (device exp's it).
        # DVE groups: 1/64 at [e, dbit_e], 0 fill (device scales ge by wv).
        isdve = np.isin(dtile[me] // GRP, list(DVE_GROUPS))
        stf = np.full((EPAD, 128), PAD_EW, np.float32)
        for g in DVE_GROUPS:
            stf[g * GMAX * 128 : (g + 1) * GMAX * 128] = 0.0
        stf[pos, dbit[me]] = np.where(isdve, 1.0 / 64.0, ew[me])
        straw = np.ascontiguousarray(
            stf.reshape(NG, GMAX, 128, 128).transpose(0, 2, 1, 3)
        ).reshape(NG * 128, GMAX * 128).astype(ml_dtypes.float8_e4m3)

        # compact per-edge-tile ew columns (for DVE groups' wv)
        ewp = np.full((NG * GMAX, 128), PAD_EW, np.float32)
        ewp[pos // 128, pos % 128] = ew[me]
        ewp = np.ascontiguousarray(ewp.T)  # [128, NG*GMAX]

        src_pad[pos] = src[me]
        core_idx.append((src_pad, straw, ewp))

    return zrows, core_idx, net, K, loff, gsum, GMAX


_COMPILED = {}


def _build_phase1(K):
    import concourse.bass as bass
    import concourse.bacc as bacc
    from concourse import mybir, tile

    f32 = mybir.dt.float32
    bf16 = mybir.dt.bfloat16
    i32 = mybir.dt.int32
    AF = mybir.ActivationFunctionType
    ALU = mybir.AluOpType
    X = mybir.AxisListType.X

    NT = HTILES       # 49 node tiles
    CH = 13           # node tiles per pipeline chunk
    NCH = (NT + CH - 1) // CH

    nc = bacc.Bacc(None, debug=False)
    fm_ext = nc.dram_tensor("feat_my", [128, NT * D], bf16, kind="ExternalInput")
    zr_ext = nc.dram_tensor("zrow", [128, NT * K], bf16, kind="ExternalInput")
    beta_ext = nc.dram_tensor("beta128", [128, 1], f32, kind="ExternalInput")
    eps_ext = nc.dram_tensor("eps128", [128, 1], f32, kind="ExternalInput")
    g_ext = nc.dram_tensor("g64", [128, NT * D], bf16, kind="ExternalOutput")
    o0_ext = nc.dram_tensor("o0", [128, NT * D], bf16, kind="ExternalOutput")

    with tile.TileContext(nc) as tc:
        with (
            tc.tile_pool(name="pp", bufs=1) as pp,
            tc.tile_pool(name="fmp", bufs=2) as fmp,
            tc.tile_pool(name="zrp", bufs=2) as zrp,
            tc.tile_pool(name="sqp", bufs=2) as sqp,
            tc.tile_pool(name="smp", bufs=2 * 8) as smp,
            tc.tile_pool(name="gp", bufs=2) as gp,
            tc.tile_pool(name="op", bufs=2) as op,
        ):
            beta_b = pp.tile([128, 1], f32, tag="beta_b")
            nc.sync.dma_start(out=beta_b[:], in_=beta_ext[:])
            ep1_b = pp.tile([128, 1], f32, tag="ep1_b")
            nc.sync.dma_start(out=ep1_b[:], in_=eps_ext[:])
            nc.vector.tensor_scalar_add(ep1_b[:], ep1_b[:], 1.0)

            for ci in range(NCH):
                t0 = ci * CH
                nt = min(CH, NT - t0)
                fm = fmp.tile([128, CH, D], bf16, tag="fm")
                nc.sync.dma_start(
                    out=fm[:, :nt, :].rearrange("p a b -> p (a b)"),
                    in_=fm_ext[:, t0 * D : (t0 + nt) * D],
                )
                zr = zrp.tile([128, CH, K], bf16, tag="zr")
                nc.sync.dma_start(
                    out=zr[:, :nt, :].rearrange("p a b -> p (a b)"),
                    in_=zr_ext[:, t0 * K : (t0 + nt) * K],
                )

                # z = sum_k exp(beta * zrow_k);  izn = 64/z
                zx = zrp.tile([128, CH, K], bf16, tag="zx")
                nc.scalar.activation(
                    zx[:, :nt, :].rearrange("p a b -> p (a b)"),
                    zr[:, :nt, :].rearrange("p a b -> p (a b)"),
                    AF.Exp,
                    scale=beta_b[:],
                )
                z = smp.tile([128, CH], f32, tag="z")
                nc.vector.tensor_reduce(z[:, :nt], zx[:, :nt, :], X, ALU.add)
                izn = smp.tile([128, CH], f32, tag="izn")
                nc.vector.reciprocal(izn[:, :nt], z[:, :nt])
                nc.vector.tensor_scalar(
                    izn[:, :nt], izn[:, :nt], 64.0, None, op0=ALU.mult
                )

                # ss = ||feat||^2 per node (bf16 squares -> 2x mode)
                sq = sqp.tile([128, CH, D], bf16, tag="sq")
                nc.vector.tensor_tensor(
                    sq[:, :nt, :].rearrange("p a b -> p (a b)"),
                    fm[:, :nt, :].rearrange("p a b -> p (a b)"),
                    fm[:, :nt, :].rearrange("p a b -> p (a b)"),
                    ALU.mult,
                )
                ss = smp.tile([128, CH], f32, tag="ss")
                nc.vector.tensor_reduce(ss[:, :nt], sq[:, :nt, :], X, ALU.add)

                # rr = 1/sqrt(ss): bit-hack + two Newton steps
                y0 = smp.tile([128, CH], f32, tag="y0")
                nc.vector.tensor_scalar(
                    y0[:, :nt].bitcast(i32), ss[:, :nt].bitcast(i32),
                    1, -1, op0=ALU.arith_shift_right, op1=ALU.bitwise_xor,
                )
                nc.vector.tensor_scalar(
                    y0[:, :nt].bitcast(i32), y0[:, :nt].bitcast(i32),
                    0x5F3759E0, None, op0=ALU.add,
                )
                u = smp.tile([128, CH], f32, tag="u")
                for _ in range(2):
                    nc.vector.tensor_tensor(
                        u[:, :nt], y0[:, :nt], y0[:, :nt], ALU.mult
                    )
                    nc.vector.tensor_tensor(
                        u[:, :nt], u[:, :nt], ss[:, :nt], ALU.mult
                    )
                    nc.vector.tensor_scalar(
                        u[:, :nt], u[:, :nt], -0.5, 1.5, op0=ALU.mult, op1=ALU.add
                    )
                    nc.vector.tensor_tensor(
                        y0[:, :nt], y0[:, :nt], u[:, :nt], ALU.mult
                    )

                # rz = 64*rr/z;  g64 = feat*rz (DVE);  o0 = (1+eps)*feat (ACT)
                rz = smp.tile([128, CH], f32, tag="rz")
                nc.vector.tensor_tensor(rz[:, :nt], y0[:, :nt], izn[:, :nt], ALU.mult)
                g = gp.tile([128, CH, D], bf16, tag="g")
                rzb = rz[:, :nt].unsqueeze(2).broadcast_to([128, nt, D])
                nc.vector.tensor_tensor(g[:, :nt, :], fm[:, :nt, :], rzb, ALU.mult)
                nc.sync.dma_start(
                    out=g_ext[:, t0 * D : (t0 + nt) * D],
                    in_=g[:, :nt, :].rearrange("p a b -> p (a b)"),
                )
                o0 = op.tile([128, CH, D], bf16, tag="o0")
                nc.scalar.activation(
                    o0[:, :nt, :].rearrange("p a b -> p (a b)"),
                    fm[:, :nt, :].rearrange("p a b -> p (a b)"),
                    AF.Copy,
                    scale=ep1_b[:],
                )
                nc.sync.dma_start(
                    out=o0_ext[:, t0 * D : (t0 + nt) * D],
                    in_=o0[:, :nt, :].rearrange("p a b -> p (a b)"),
                )

    nc.finalize()
    return nc


def _build_phase2(net, loff, gsum, GMAX):
    import concourse.bass as bass
    import concourse.bacc as bacc
    from concourse import mybir, tile

    f32 = mybir.dt.float32
    bf16 = mybir.dt.bfloat16
    f8 = mybir.dt.float8e4
    AF = mybir.ActivationFunctionType
    ALU = mybir.AluOpType

    nc = bacc.Bacc(None, debug=False)
    ge_ext = nc.dram_tensor("ge", [NG * 128, GMAX * D], f8, kind="ExternalInput")
    st_ext = nc.dram_tensor("straw", [NG * 128, GMAX * 128], f8, kind="ExternalInput")
    ewp_ext = nc.dram_tensor("ewp", [128, NG * GMAX], f32, kind="ExternalInput")
    o0_ext = nc.dram_tensor("o0", [128, HTILES * D], bf16, kind="ExternalInput")
    beta_ext = nc.dram_tensor("beta128", [128, 1], f32, kind="ExternalInput")
    out_ext = nc.dram_tensor("out", [128, HTILES * D], bf16, kind="ExternalOutput")

    with tile.TileContext(nc) as tc:
        with (
            tc.tile_pool(name="persist", bufs=1) as pp,
            tc.tile_pool(name="gep", bufs=4) as gepool,
            tc.tile_pool(name="stp", bufs=4) as stpool,
            tc.tile_pool(name="stw", bufs=3) as stwpool,
            tc.tile_pool(name="mqp", bufs=3) as mqpool,
            tc.tile_pool(name="o0p", bufs=3) as o0pool,
            tc.tile_pool(name="outp", bufs=3) as opool,
            tc.tile_pool(name="hpsum", bufs=8, space="PSUM") as hpsum,
        ):
            beta_b = pp.tile([128, 1], f32, tag="beta_b")
            nc.sync.dma_start(out=beta_b[:], in_=beta_ext[:])
            bl64 = pp.tile([128, 1], f32, tag="bl64")
            nc.vector.memset(bl64[:], -LN64)

            # per-edge w for DVE groups: wv = exp(beta*ewp) (one shot)
            wv = pp.tile([128, NG * GMAX], f32, tag="wv")
            nc.sync.dma_start(out=wv[:], in_=ewp_ext[:])
            nc.scalar.activation(wv[:], wv[:], AF.Exp, scale=beta_b[:])

            for g in range(NG):
                i0 = g * GRP
                tiles = list(range(i0, min(i0 + GRP, HTILES)))
                gnh = int(gsum[g])
                dve = g in DVE_GROUPS

                straw = stpool.tile([128, GMAX, 128], f8, tag="straw")
                nc.sync.dma_start(
                    out=straw[:, :gnh, :].rearrange("p a b -> p (a b)"),
                    in_=st_ext[g * 128 : (g + 1) * 128, : gnh * 128],
                )
                ge = gepool.tile([128, GMAX, D], f8, tag="ge")
                nc.sync.dma_start(
                    out=ge[:, :gnh, :].rearrange("p a b -> p (a b)"),
                    in_=ge_ext[g * 128 : (g + 1) * 128, : gnh * D],
                )
                o0g = o0pool.tile([128, GRP, D], bf16, tag="o0g")
                nw = len(tiles)
                nc.sync.dma_start(
                    out=o0g[:, :nw, :].rearrange("p a b -> p (a b)"),
                    in_=o0_ext[:, i0 * D : (i0 + nw) * D],
                )

                if dve:
                    # msgq = ge * wv (broadcast along D); lhsT = onehot/64 fp8
                    msgq = mqpool.tile([128, GMAX, D], bf16, tag="msgq")
                    wb = (
                        wv[:, g * GMAX : g * GMAX + gnh]
                        .unsqueeze(2)
                        .broadcast_to([128, gnh, D])
                    )
                    nc.vector.tensor_tensor(
                        msgq[:, :gnh, :], ge[:, :gnh, :], wb, ALU.mult
                    )
                    lhs_t, rhs_t = straw, msgq
                else:
                    # stw = exp(beta*straw - ln64): w_e/64 one-hot-placed
                    stw = stwpool.tile([128, GMAX, 128], bf16, tag="stw")
                    nc.scalar.activation(
                        stw[:, :gnh, :].rearrange("p a b -> p (a b)"),
                        straw[:, :gnh, :].rearrange("p a b -> p (a b)"),
                        AF.Exp,
                        bias=bl64[:],
                        scale=beta_b[:],
                    )
                    lhs_t, rhs_t = stw, ge

                og = opool.tile([128, GRP, D], bf16, tag="og")
                for i in tiles:
                    nh, lo = int(net[i]), int(loff[i])
                    hp = hpsum.tile([128, D], f32, tag="hp")
                    for t in range(nh):
                        nc.tensor.matmul(
                            hp[:],
                            lhs_t[:, lo + t, :],
                            rhs_t[:, lo + t, :],
                            start=(t == 0),
                            stop=(t == nh - 1),
                        )
                    nc.vector.tensor_tensor(
                        og[:, i - i0, :], o0g[:, i - i0, :], hp[:], ALU.add
                    )
                nc.sync.dma_start(
                    out=out_ext[:, i0 * D : (i0 + nw) * D],
                    in_=og[:, :nw, :].rearrange("p a b -> p (a b)"),
                )

    nc.finalize()
    return nc


def kernel(feat, edge_weight, src, dst, beta, eps):
    from concourse.bass_utils import run_bass_kernel_spmd
    import ml_dtypes

    feat = np.asarray(feat, dtype=np.float32)
    ew = np.asarray(edge_weight, dtype=np.float32)
    beta = np.asarray(beta, dtype=np.float32)
    eps = np.asarray(eps, dtype=np.float32)

    zrows, core_idx, net, K, loff, gsum, GMAX = _host_prep(src, dst, ew)

    key = (K, GMAX, tuple(int(x) for x in net))
    if key not in _COMPILED:
        _COMPILED[key] = (
            _build_phase1(K),
            _build_phase2(net, loff, gsum, GMAX),
        )
    nc1, nc2 = _COMPILED[key]

    beta128 = np.ascontiguousarray(np.broadcast_to(beta.reshape(1, 1), (128, 1)))
    eps128 = np.ascontiguousarray(np.broadcast_to(eps.reshape(1, 1), (128, 1)))

    # ---------------- phase 1: per-node g64, o0 ----------------
    in1 = []
    for c in range(NCORES):
        fmp = np.zeros((SHP, D), np.float32)
        fmp[:SH] = feat[c * SH : (c + 1) * SH]
        fmt = np.ascontiguousarray(
            fmp.reshape(HTILES, 128, D).transpose(1, 0, 2)
        ).reshape(128, HTILES * D).astype(ml_dtypes.bfloat16)
        in1.append(
            {"feat_my": fmt, "zrow": zrows[c], "beta128": beta128,
             "eps128": eps128}
        )

    res1 = run_bass_kernel_spmd(nc1, in1, core_ids=list(range(NCORES)))
    gfull = np.empty((N + 1, D), dtype=ml_dtypes.float8_e4m3)
    o0s = []
    for c in range(NCORES):
        gc = np.asarray(res1.results[c]["g64"]).reshape(128, HTILES, D)
        gfull[c * SH : (c + 1) * SH] = (
            gc.transpose(1, 0, 2).reshape(SHP, D)[:SH].astype(ml_dtypes.float8_e4m3)
        )
        o0s.append(np.asarray(res1.results[c]["o0"]))
    gfull[N] = 0  # pad row

    # ---------------- host gather of g64[src_e] ----------------
    in2 = []
    for c in range(NCORES):
        src_pad, straw, ewp = core_idx[c]
        ge = np.ascontiguousarray(
            gfull[src_pad].reshape(NG, GMAX, 128, D).transpose(0, 2, 1, 3)
        ).reshape(NG * 128, GMAX * D)
        in2.append(
            {"ge": ge, "straw": straw, "ewp": ewp, "o0": o0s[c],
             "beta128": beta128}
        )

    res2 = run_bass_kernel_spmd(nc2, in2, core_ids=list(range(NCORES)))
    out = np.empty((N, D), np.float32)
    for c in range(NCORES):
        oc = np.asarray(res2.results[c]["out"]).reshape(128, HTILES, D)
        out[c * SH : (c + 1) * SH] = (
            oc.transpose(1, 0, 2).reshape(SHP, D)[:SH].astype(np.float32)
        )
    return out
